# revision 1
# baseline (speedup 1.0000x reference)
"""Cross-attention block (B=16, N=4096 queries, M=77 keys, 8 heads x 64) on 8 trn2 cores.

Sharding: data-parallel over batch; each core gets 2 batches, full weights.

Per-core dataflow (matmuls bf16 in / fp32 psum):
  x -> bf16 staging copy in DRAM (gpsimd cast DMA), per 512-token chunk
  xbar-transpose-loaded as xT [feat, tok].
  qT = Wq.T @ xT                   (weight-stationary)
  per head h: sT = kT_h.T @ qT_h -> exp(sT/8) -> E[77, H, tok]
  denominators: 8 indicator-matmuls accumulate colsum(E_h) into psum [8, tok],
  reciprocal_approx_fast, bounce through DRAM to broadcast across partitions.
  per head-pair: O.T = v_h.T @ E_h into psum halves; aT = O.T * recip (DVE)
  out = aT.T @ Wo + bo             (aT chunks stationary -> token-major out)
"""

import numpy as np

import concourse.bass as bass
import concourse.mybir as mybir
import concourse.tile as tile
from concourse import bacc
from concourse._compat import with_exitstack
from concourse.bass_utils import run_bass_kernel_spmd
from concourse.masks import make_identity
from contextlib import ExitStack

N_CORES = 8
B, N, FEAT, CD = 16, 4096, 512, 768
M = 77          # cond tokens
H, DH = 8, 64
DA = H * DH     # 512
BP = B // N_CORES   # batches per core
TC = 512            # token chunk
NT = N // TC        # chunks per batch
SUB = TC // 128     # 128-token subtiles per chunk
KC = FEAT // 128    # x feature chunks
CC = CD // 128      # cond feature chunks
MC = DA // 128      # d_attn chunks
HPAIRS = H // 2

F32 = mybir.dt.float32
BF16 = mybir.dt.bfloat16
EXP = mybir.ActivationFunctionType.Exp


@with_exitstack
def _body(ctx: ExitStack, tc: tile.TileContext, x, x_bf, cond, Wq, Wk, Wv, Wo, bo, out):
    nc = tc.nc

    wpool = ctx.enter_context(tc.tile_pool(name="wpool", bufs=1))
    Wq_bf = wpool.tile([128, KC, DA], BF16, tag="wq")
    Wk_bf = wpool.tile([128, CC, DA], BF16, tag="wk")
    Wv_bf = wpool.tile([128, CC, DA], BF16, tag="wv")
    Wo_bf = wpool.tile([128, MC, FEAT], BF16, tag="wo")
    bo_bc = wpool.tile([128, FEAT], F32, tag="bo")
    ident = wpool.tile([128, 128], F32, tag="ident")
    # 0/1 picker: col 8 is ones; colpick[:, 8-h : 16-h] selects head h
    colpick = wpool.tile([128, 17], BF16, tag="colpick")

    for k in range(KC):
        nc.gpsimd.dma_start(out=Wq_bf[:, k, :], in_=Wq[128 * k : 128 * (k + 1), :])
    for c in range(CC):
        nc.gpsimd.dma_start(out=Wk_bf[:, c, :], in_=Wk[128 * c : 128 * (c + 1), :])
        nc.gpsimd.dma_start(out=Wv_bf[:, c, :], in_=Wv[128 * c : 128 * (c + 1), :])
    for m in range(MC):
        nc.gpsimd.dma_start(out=Wo_bf[:, m, :], in_=Wo[128 * m : 128 * (m + 1), :])
    bo_bcast_ap = bass.AP(tensor=bo.tensor, offset=bo.offset, ap=[[0, 128], *bo.ap])
    nc.gpsimd.dma_start(out=bo_bc[:, :], in_=bo_bcast_ap)
    make_identity(nc, ident)
    nc.gpsimd.memset(colpick[:, :], 0.0)
    nc.gpsimd.memset(colpick[:, 8:9], 1.0)

    # bf16 staging copy of x (transpose-loads below need a 2-byte dtype)
    for b in range(BP):
        for q in range(4):
            nc.gpsimd.dma_start(
                out=x_bf[b, 1024 * q : 1024 * (q + 1), :],
                in_=x[b, 1024 * q : 1024 * (q + 1), :],
            )

    bpool = ctx.enter_context(tc.tile_pool(name="bpool", bufs=2))
    tpool = ctx.enter_context(tc.tile_pool(name="tpool", bufs=3))
    qpool = ctx.enter_context(tc.tile_pool(name="qpool", bufs=3))
    epool = ctx.enter_context(tc.tile_pool(name="epool", bufs=3))
    rpool = ctx.enter_context(tc.tile_pool(name="rpool", bufs=6))
    apool = ctx.enter_context(tc.tile_pool(name="apool", bufs=3))
    opool = ctx.enter_context(tc.tile_pool(name="opool", bufs=4))

    dpool = ctx.enter_context(tc.tile_pool(name="dpool", bufs=2, space="DRAM"))

    psq = ctx.enter_context(tc.tile_pool(name="psq", bufs=2, space="PSUM"))
    pss = ctx.enter_context(tc.tile_pool(name="pss", bufs=2, space="PSUM"))
    pso = ctx.enter_context(tc.tile_pool(name="pso", bufs=2, space="PSUM"))
    psu = ctx.enter_context(tc.tile_pool(name="psu", bufs=1, space="PSUM"))
    psm = ctx.enter_context(tc.tile_pool(name="psm", bufs=1, space="PSUM"))

    for b in range(BP):
        # cond[b] -> cond.T (PE transpose) -> K/V projections
        cond_sb = bpool.tile([128, CD], F32, tag="cond")
        nc.sync.dma_start(out=cond_sb[:M, :], in_=cond[b, :, :])
        condT = bpool.tile([128, CC, M], BF16, tag="condT")
        for c in range(CC):
            ps = pss.tile([128, TC], F32, tag="pss")
            nc.tensor.matmul(
                ps[:128, :M],
                cond_sb[:M, 128 * c : 128 * (c + 1)],
                ident[:M, :M],
                is_transpose=True,
            )
            nc.scalar.copy(condT[:, c, :], ps[:128, :M])

        # kT[d_attn, M] = Wk.T @ cond.T
        kT = bpool.tile([128, MC, M], BF16, tag="kT")
        for m in range(MC):
            pk = psq.tile([128, TC], F32, tag="psq")
            for c in range(CC):
                nc.tensor.matmul(
                    pk[:, :M],
                    Wk_bf[:, c, 128 * m : 128 * (m + 1)],
                    condT[:, c, :],
                    start=(c == 0),
                    stop=(c == CC - 1),
                )
            nc.scalar.copy(kT[:, m, :], pk[:, :M])

        # v[M, d_attn] = cond @ Wv  (cond.T is the stationary operand)
        pv = pso.tile([128, TC], F32, tag="pso")
        for c in range(CC):
            nc.tensor.matmul(
                pv[:M, :],
                condT[:, c, :],
                Wv_bf[:, c, :],
                start=(c == 0),
                stop=(c == CC - 1),
            )
        v_bf = bpool.tile([128, DA], BF16, tag="v")
        nc.scalar.copy(v_bf[:M, :], pv[:M, :])

        for t in range(NT):
            tok0 = t * TC
            # xT[feat, tok] via xbar transpose straight from the bf16 staging copy
            xT = tpool.tile([128, KC, TC], BF16, tag="xT")
            for k in range(KC):
                nc.sync.dma_start(
                    out=xT[:, k, :],
                    in_=x_bf[b, tok0 : tok0 + TC, 128 * k : 128 * (k + 1)],
                    transpose=True,
                )

            # qT[d_attn, tok] = Wq.T @ xT
            qT = qpool.tile([128, MC, TC], BF16, tag="qT")
            for m in range(MC):
                pq = psq.tile([128, TC], F32, tag="psq")
                for k in range(KC):
                    nc.tensor.matmul(
                        pq,
                        Wq_bf[:, k, 128 * m : 128 * (m + 1)],
                        xT[:, k, :],
                        start=(k == 0),
                        stop=(k == KC - 1),
                    )
                nc.scalar.copy(qT[:, m, :], pq)

            # scores + exp per head; also accumulate per-head colsums on PE
            E = epool.tile([128, H, TC], BF16, tag="E")
            sm = psm.tile([8, TC], F32, tag="psm")
            for h in range(H):
                hp, r = h // 2, 64 * (h % 2)
                ps = pss.tile([128, TC], F32, tag="pss")
                nc.tensor.matmul(
                    ps[:M, :],
                    kT[r : r + 64, hp, :],
                    qT[r : r + 64, hp, :],
                    start=True,
                    stop=True,
                )
                nc.scalar.activation(E[:M, h, :], ps[:M, :], func=EXP, scale=DH**-0.5)
                nc.tensor.matmul(
                    sm,
                    colpick[:M, 8 - h : 16 - h],
                    E[:M, h, :],
                    start=(h == 0),
                    stop=(h == H - 1),
                )

            # 1/sums, bounced through DRAM to broadcast rows across partitions
            r8 = rpool.tile([8, TC], F32, tag="r8")
            nc.vector.reciprocal_approx_fast(out=r8[:8, :], in_=sm[:8, :])
            r8d = dpool.tile([8, TC], F32, tag="r8d")
            nc.gpsimd.dma_start(out=r8d[:, :], in_=r8[:8, :])
            rss = []
            for hp in range(HPAIRS):
                rs = rpool.tile([128, TC], F32, tag="rs")
                bcast_src = r8d[2 * hp : 2 * hp + 2, :]
                bcast_ap = bass.AP(
                    tensor=bcast_src.tensor,
                    offset=bcast_src.offset,
                    ap=[bcast_src.ap[0], [0, 64], *bcast_src.ap[1:]],
                )
                nc.gpsimd.dma_start(out=rs[:, :], in_=bcast_ap)
                rss.append(rs)

            # attn @ v, normalized at copyback: aT[d_attn, tok]
            aT = apool.tile([128, MC, TC], BF16, tag="aT")
            for hp in range(HPAIRS):
                po = pso.tile([128, TC], F32, tag="pso")
                nc.tensor.matmul(
                    po[0:64, :],
                    v_bf[:M, 128 * hp : 128 * hp + 64],
                    E[:M, 2 * hp, :],
                    start=True,
                    stop=True,
                )
                nc.tensor.matmul(
                    po[64:128, :],
                    v_bf[:M, 128 * hp + 64 : 128 * (hp + 1)],
                    E[:M, 2 * hp + 1, :],
                    start=True,
                    stop=True,
                )
                ao = rpool.tile([128, TC], F32, tag="ao")
                nc.scalar.copy(ao[:, :], po[:, :])
                nc.vector.tensor_mul(aT[:, hp, :], ao[:, :], rss[hp][:, :])

            # out = aT.T @ Wo + bo  (aT chunks stationary -> token-major psum)
            for s in range(SUB):
                pu = psu.tile([128, FEAT], F32, tag="psu")
                for m in range(MC):
                    nc.tensor.matmul(
                        pu,
                        aT[:, m, 128 * s : 128 * (s + 1)],
                        Wo_bf[:, m, :],
                        start=(m == 0),
                        stop=(m == MC - 1),
                    )
                osb = opool.tile([128, FEAT], F32, tag="osb")
                nc.vector.tensor_add(osb, pu, bo_bc)
                nc.sync.dma_start(
                    out=out[b, tok0 + 128 * s : tok0 + 128 * (s + 1), :], in_=osb
                )


def build():
    nc = bacc.Bacc(
        "TRN2", target_bir_lowering=False, debug=False, num_devices=N_CORES
    )
    x = nc.dram_tensor("x", [BP, N, FEAT], F32, kind="ExternalInput").ap()
    cond = nc.dram_tensor("cond", [BP, M, CD], F32, kind="ExternalInput").ap()
    Wq = nc.dram_tensor("Wq", [FEAT, DA], F32, kind="ExternalInput").ap()
    Wk = nc.dram_tensor("Wk", [CD, DA], F32, kind="ExternalInput").ap()
    Wv = nc.dram_tensor("Wv", [CD, DA], F32, kind="ExternalInput").ap()
    Wo = nc.dram_tensor("Wo", [DA, FEAT], F32, kind="ExternalInput").ap()
    bo = nc.dram_tensor("bo", [FEAT], F32, kind="ExternalInput").ap()
    out = nc.dram_tensor("out", [BP, N, FEAT], F32, kind="ExternalOutput").ap()
    x_bf = nc.dram_tensor("x_bf16_stage", [BP, N, FEAT], BF16).ap()
    with tile.TileContext(nc) as tc:
        _body(tc, x, x_bf, cond, Wq, Wk, Wv, Wo, bo, out)
    nc.compile()
    return nc


_NC = None


def kernel(x, cond, Wq, Wk, Wv, Wo, bo, _trace=False):
    global _NC
    if _NC is None:
        _NC = build()
    shared = {
        "Wq": np.asarray(Wq, np.float32),
        "Wk": np.asarray(Wk, np.float32),
        "Wv": np.asarray(Wv, np.float32),
        "Wo": np.asarray(Wo, np.float32),
        "bo": np.asarray(bo, np.float32),
    }
    in_maps = [
        {
            "x": np.ascontiguousarray(x[BP * i : BP * (i + 1)], dtype=np.float32),
            "cond": np.ascontiguousarray(cond[BP * i : BP * (i + 1)], dtype=np.float32),
            **shared,
        }
        for i in range(N_CORES)
    ]
    res = run_bass_kernel_spmd(_NC, in_maps, list(range(N_CORES)), trace=_trace)
    out = np.concatenate([r["out"] for r in res.results], axis=0)
    if _trace:
        kernel.last_exec_time_ns = res.exec_time_ns
        kernel.last_results = res
    return out



# revision 9
# speedup vs baseline: 1.1185x; 1.1185x over previous
"""Cross-attention block (B=16, N=4096 queries, M=77 keys, 8 heads x 64) on 8 trn2 cores.

Sharding: data-parallel over batch; each core gets 2 batches, full weights.

Per-core dataflow (matmuls bf16 in / fp32 psum), tuned to keep the PE warm:
  x -> bf16 staging copy in DRAM (gpsimd cast DMA), per 512-token chunk,
  xbar-transpose-loaded as xT [feat, tok].
  qT = Wq.T @ xT                    (weight-stationary)
  per head-pair: two row-tiled score matmuls into a 2-bank psum tile,
  one batched exp -> E[77, H, tok]; indicator-matmul colsums -> psum [8, tok].
  reciprocal (bf16) broadcast across partitions via a K=8 indicator matmul
  (no DRAM bounce); aT = (v_h.T @ E_h) * recip on DVE.
  out = aT.T @ Wo + bo, bias pre-loaded into psum via a K=1 ones matmul.
"""

import numpy as np

import concourse.bass as bass
import concourse.mybir as mybir
import concourse.tile as tile
from concourse import bacc
from concourse._compat import with_exitstack
from concourse.bass_utils import run_bass_kernel_spmd
from concourse.masks import make_identity
from contextlib import ExitStack

N_CORES = 8
B, N, FEAT, CD = 16, 4096, 512, 768
M = 77          # cond tokens
H, DH = 8, 64
DA = H * DH     # 512
BP = B // N_CORES   # batches per core
TC = 512            # token chunk
NT = N // TC        # chunks per batch
SUB = TC // 128     # 128-token subtiles per chunk
KC = FEAT // 128    # x feature chunks
CC = CD // 128      # cond feature chunks
MC = DA // 128      # d_attn chunks
HPAIRS = H // 2

F32 = mybir.dt.float32
BF16 = mybir.dt.bfloat16
EXP = mybir.ActivationFunctionType.Exp


@with_exitstack
def _body(
    ctx: ExitStack, tc: tile.TileContext, x, x_bf, cond, Wq, Wk, Wv, Wo, bo, indb_d, out
):
    nc = tc.nc

    wpool = ctx.enter_context(tc.tile_pool(name="wpool", bufs=1))
    Wq_bf = wpool.tile([128, KC, DA], BF16, tag="wq")
    Wk_bf = wpool.tile([128, CC, DA], BF16, tag="wk")
    Wv_bf = wpool.tile([128, CC, DA], BF16, tag="wv")
    Wo_bf = wpool.tile([128, MC, FEAT], BF16, tag="wo")
    bo_row = wpool.tile([1, FEAT], BF16, tag="bo")
    ones1 = wpool.tile([1, 128], BF16, tag="ones1")
    ident = wpool.tile([128, 128], F32, tag="ident")
    # 0/1 picker: col 8 is ones; colpick[:, 8-h : 16-h] selects head h
    colpick = wpool.tile([128, 17], BF16, tag="colpick")


    for k in range(KC):
        nc.gpsimd.dma_start(out=Wq_bf[:, k, :], in_=Wq[128 * k : 128 * (k + 1), :])
    for c in range(CC):
        nc.gpsimd.dma_start(out=Wk_bf[:, c, :], in_=Wk[128 * c : 128 * (c + 1), :])
        nc.gpsimd.dma_start(out=Wv_bf[:, c, :], in_=Wv[128 * c : 128 * (c + 1), :])
    for m in range(MC):
        nc.gpsimd.dma_start(out=Wo_bf[:, m, :], in_=Wo[128 * m : 128 * (m + 1), :])
    bo_row_ap = bass.AP(tensor=bo.tensor, offset=bo.offset, ap=[[0, 1], *bo.ap])
    nc.gpsimd.dma_start(out=bo_row[:, :], in_=bo_row_ap)
    # per head-pair hp: indb[2hp, hp, 0:64]=1, indb[2hp+1, hp, 64:128]=1
    indb = wpool.tile([8, HPAIRS, 128], BF16, tag="indb")
    nc.sync.dma_start(out=indb[:, :, :], in_=indb_d[:, :, :])
    make_identity(nc, ident)
    nc.gpsimd.memset(ones1[:, :], 1.0)
    nc.gpsimd.memset(colpick[:, :], 0.0)
    nc.gpsimd.memset(colpick[:, 8:9], 1.0)

    # bf16 staging copy of x (transpose-loads below need a 2-byte dtype),
    # issued chunk-by-chunk in consumption order.
    for b in range(BP):
        for t in range(NT):
            nc.gpsimd.dma_start(
                out=x_bf[b, TC * t : TC * (t + 1), :],
                in_=x[b, TC * t : TC * (t + 1), :],
            )

    bpool = ctx.enter_context(tc.tile_pool(name="bpool", bufs=2))
    tpool = ctx.enter_context(tc.tile_pool(name="tpool", bufs=3))
    qpool = ctx.enter_context(tc.tile_pool(name="qpool", bufs=2))
    epool = ctx.enter_context(tc.tile_pool(name="epool", bufs=2))
    rpool = ctx.enter_context(tc.tile_pool(name="rpool", bufs=2))
    apool = ctx.enter_context(tc.tile_pool(name="apool", bufs=2))
    opool = ctx.enter_context(tc.tile_pool(name="opool", bufs=3))

    # PSUM: psQ 2 + psS 2 (one 2-bank tile) + psC 1 + psV 1 + psO 2 = 8 banks
    psQ = ctx.enter_context(tc.tile_pool(name="psQ", bufs=2, space="PSUM"))
    psS = ctx.enter_context(tc.tile_pool(name="psS", bufs=1, space="PSUM"))
    psC = ctx.enter_context(tc.tile_pool(name="psC", bufs=1, space="PSUM"))
    psV = ctx.enter_context(tc.tile_pool(name="psV", bufs=1, space="PSUM"))
    psO = ctx.enter_context(tc.tile_pool(name="psO", bufs=2, space="PSUM"))

    for b in range(BP):
        # cond[b] -> cond.T (PE transpose) -> K/V projections
        cond_sb = bpool.tile([128, CD], F32, tag="cond")
        nc.sync.dma_start(out=cond_sb[:M, :], in_=cond[b, :, :])
        condT = bpool.tile([128, CC, M], BF16, tag="condT")
        for c in range(CC):
            ps = psQ.tile([128, TC], F32, tag="psQ")
            nc.tensor.matmul(
                ps[:128, :M],
                cond_sb[:M, 128 * c : 128 * (c + 1)],
                ident[:M, :M],
                is_transpose=True,
            )
            nc.scalar.copy(condT[:, c, :], ps[:128, :M])

        # kT[d_attn, M] = Wk.T @ cond.T
        kT = bpool.tile([128, MC, M], BF16, tag="kT")
        for m in range(MC):
            pk = psQ.tile([128, TC], F32, tag="psQ")
            for c in range(CC):
                nc.tensor.matmul(
                    pk[:, :M],
                    Wk_bf[:, c, 128 * m : 128 * (m + 1)],
                    condT[:, c, :],
                    start=(c == 0),
                    stop=(c == CC - 1),
                )
            nc.scalar.copy(kT[:, m, :], pk[:, :M])

        # v[M, d_attn] = cond @ Wv  (cond.T is the stationary operand)
        pv = psQ.tile([128, TC], F32, tag="psQ")
        for c in range(CC):
            nc.tensor.matmul(
                pv[:M, :],
                condT[:, c, :],
                Wv_bf[:, c, :],
                start=(c == 0),
                stop=(c == CC - 1),
            )
        v_bf = bpool.tile([128, DA], BF16, tag="v")
        nc.scalar.copy(v_bf[:M, :], pv[:M, :])

        for t in range(NT):
            tok0 = t * TC
            # xT[feat, tok] via xbar transpose straight from the bf16 staging copy
            xT = tpool.tile([128, KC, TC], BF16, tag="xT")
            for k in range(KC):
                nc.sync.dma_start(
                    out=xT[:, k, :],
                    in_=x_bf[b, tok0 : tok0 + TC, 128 * k : 128 * (k + 1)],
                    transpose=True,
                )

            # qT[d_attn, tok] = Wq.T @ xT
            qT = qpool.tile([128, MC, TC], BF16, tag="qT")
            for m in range(MC):
                pq = psQ.tile([128, TC], F32, tag="psQ")
                for k in range(KC):
                    nc.tensor.matmul(
                        pq,
                        Wq_bf[:, k, 128 * m : 128 * (m + 1)],
                        xT[:, k, :],
                        start=(k == 0),
                        stop=(k == KC - 1),
                    )
                nc.vector.tensor_copy(qT[:, m, :], pq)

            # scores + exp per head-pair (row-tiled concurrent matmuls into a
            # 2-bank psum tile, one batched exp); per-head colsums on PE
            E = epool.tile([128, H, TC], BF16, tag="E")
            sm = psC.tile([8, TC], F32, tag="cd")
            for hp in range(HPAIRS):
                pse = psS.tile([128, 2, TC], F32, tag="pse")
                for r in range(2):
                    h = 2 * hp + r
                    nc.tensor.matmul(
                        pse[:M, r, :],
                        kT[64 * r : 64 * r + 64, hp, :],
                        qT[64 * r : 64 * r + 64, hp, :],
                        start=True,
                        stop=True,
                    )
                nc.scalar.activation(
                    E[:M, 2 * hp : 2 * hp + 2, :],
                    pse[:M, :, :],
                    func=EXP,
                    scale=DH**-0.5,
                )
                for r in range(2):
                    h = 2 * hp + r
                    nc.tensor.matmul(
                        sm,
                        colpick[:M, 8 - h : 16 - h],
                        E[:M, h, :],
                        start=(h == 0),
                        stop=(h == H - 1),
                    )

            # 1/sums (cast to bf16), broadcast across partitions via K=8 matmul
            r8 = rpool.tile([8, TC], F32, tag="r8")
            nc.vector.reciprocal_approx_fast(out=r8[:8, :], in_=sm[:8, :])
            r8b = rpool.tile([8, TC], BF16, tag="r8b")
            nc.vector.tensor_copy(r8b[:8, :], r8[:8, :])

            # attn @ v, normalized at copyback: aT[d_attn, tok]
            aT = apool.tile([128, MC, TC], BF16, tag="aT")
            for hp in range(HPAIRS):
                psb = psC.tile([128, TC], F32, tag="cd")
                nc.tensor.matmul(
                    psb, indb[:8, hp, :], r8b[:8, :], start=True, stop=True
                )
                rs = rpool.tile([128, TC], F32, tag="rs")
                nc.vector.tensor_copy(rs, psb)
                po = psV.tile([128, TC], F32, tag="po")
                nc.tensor.matmul(
                    po[0:64, :],
                    v_bf[:M, 128 * hp : 128 * hp + 64],
                    E[:M, 2 * hp, :],
                    start=True,
                    stop=True,
                )
                nc.tensor.matmul(
                    po[64:128, :],
                    v_bf[:M, 128 * hp + 64 : 128 * (hp + 1)],
                    E[:M, 2 * hp + 1, :],
                    start=True,
                    stop=True,
                )
                nc.vector.tensor_mul(aT[:, hp, :], po[:, :], rs[:, :])

            # out = aT.T @ Wo + bo  (bias enters psum via a K=1 ones matmul)
            for s in range(SUB):
                pu = psO.tile([128, FEAT], F32, tag="pu")
                nc.tensor.matmul(
                    pu, ones1[0:1, :], bo_row[0:1, :], start=True, stop=False
                )
                for m in range(MC):
                    nc.tensor.matmul(
                        pu,
                        aT[:, m, 128 * s : 128 * (s + 1)],
                        Wo_bf[:, m, :],
                        start=False,
                        stop=(m == MC - 1),
                    )
                osb = opool.tile([128, FEAT], F32, tag="osb")
                nc.vector.tensor_copy(osb, pu)
                nc.scalar.dma_start(
                    out=out[b, tok0 + 128 * s : tok0 + 128 * (s + 1), :], in_=osb
                )


def build():
    nc = bacc.Bacc(
        "TRN2", target_bir_lowering=False, debug=False, num_devices=N_CORES
    )
    x = nc.dram_tensor("x", [BP, N, FEAT], F32, kind="ExternalInput").ap()
    cond = nc.dram_tensor("cond", [BP, M, CD], F32, kind="ExternalInput").ap()
    Wq = nc.dram_tensor("Wq", [FEAT, DA], F32, kind="ExternalInput").ap()
    Wk = nc.dram_tensor("Wk", [CD, DA], F32, kind="ExternalInput").ap()
    Wv = nc.dram_tensor("Wv", [CD, DA], F32, kind="ExternalInput").ap()
    Wo = nc.dram_tensor("Wo", [DA, FEAT], F32, kind="ExternalInput").ap()
    bo = nc.dram_tensor("bo", [FEAT], F32, kind="ExternalInput").ap()
    indb_d = nc.dram_tensor("indb", [8, HPAIRS, 128], BF16, kind="ExternalInput").ap()
    out = nc.dram_tensor("out", [BP, N, FEAT], F32, kind="ExternalOutput").ap()
    x_bf = nc.dram_tensor("x_bf16_stage", [BP, N, FEAT], BF16).ap()
    with tile.TileContext(nc) as tc:
        _body(tc, x, x_bf, cond, Wq, Wk, Wv, Wo, bo, indb_d, out)
    nc.compile()
    return nc


_NC = None


def kernel(x, cond, Wq, Wk, Wv, Wo, bo, _trace=False):
    global _NC
    if _NC is None:
        _NC = build()
    import ml_dtypes

    indb_np = np.zeros((8, HPAIRS, 128), dtype=ml_dtypes.bfloat16)
    for hp in range(HPAIRS):
        indb_np[2 * hp, hp, 0:64] = 1
        indb_np[2 * hp + 1, hp, 64:128] = 1
    shared = {
        "Wq": np.asarray(Wq, np.float32),
        "Wk": np.asarray(Wk, np.float32),
        "Wv": np.asarray(Wv, np.float32),
        "Wo": np.asarray(Wo, np.float32),
        "bo": np.asarray(bo, np.float32),
        "indb": indb_np,
    }
    in_maps = [
        {
            "x": np.ascontiguousarray(x[BP * i : BP * (i + 1)], dtype=np.float32),
            "cond": np.ascontiguousarray(cond[BP * i : BP * (i + 1)], dtype=np.float32),
            **shared,
        }
        for i in range(N_CORES)
    ]
    res = run_bass_kernel_spmd(_NC, in_maps, list(range(N_CORES)), trace=_trace)
    out = np.concatenate([r["out"] for r in res.results], axis=0)
    if _trace:
        kernel.last_exec_time_ns = res.exec_time_ns
        kernel.last_results = res
    return out


# revision 12
# speedup vs baseline: 1.2187x; 1.0895x over previous
"""Cross-attention block (B=16, N=4096 queries, M=77 keys, 8 heads x 64) on 8 trn2 cores.

Sharding: data-parallel over batch; each core gets 2 batches, full weights.

Per-core dataflow (matmuls bf16 in / fp32 psum), software-pipelined so the PE
always has independent work (HAM stays warm):
  x -> bf16 staging copy in DRAM (gpsimd cast DMA), xbar-transposed per chunk
  into xT [feat, tok];  qT = Wq.T @ xT.
  Per chunk g the emission order is: scores(g) -> qT(g+1) -> colsums(g) ->
  recip -> bcast/attnv/normalize(g) -> out(g), so qT(g+1) matmuls fill the
  PE while the scalar engine runs the exps of chunk g.
  Softmax denominators via indicator-matmul colsums; reciprocals broadcast
  across partitions with a K=8 indicator matmul (all on-chip, no DRAM bounce);
  aT = (v_h.T @ E_h) * recip with a both-PSUM DVE multiply.
  out = aT.T @ Wo + bo (bias added during the PSUM->SBUF copy on DVE).
"""

import numpy as np

import concourse.bass as bass
import concourse.mybir as mybir
import concourse.tile as tile
from concourse import bacc
from concourse._compat import with_exitstack
from concourse.bass_utils import run_bass_kernel_spmd
from concourse.masks import make_identity
from contextlib import ExitStack

N_CORES = 8
B, N, FEAT, CD = 16, 4096, 512, 768
M = 77          # cond tokens
H, DH = 8, 64
DA = H * DH     # 512
BP = B // N_CORES   # batches per core
TC = 512            # token chunk
NT = N // TC        # chunks per batch
NG = BP * NT        # chunks per core
SUB = TC // 128     # 128-token subtiles per chunk
KC = FEAT // 128    # x feature chunks
CC = CD // 128      # cond feature chunks
MC = DA // 128      # d_attn chunks
HPAIRS = H // 2

F32 = mybir.dt.float32
BF16 = mybir.dt.bfloat16
EXP = mybir.ActivationFunctionType.Exp


@with_exitstack
def _body(
    ctx: ExitStack, tc: tile.TileContext, x, x_bf, cond, Wq, Wk, Wv, Wo, bo, indb_d, out
):
    nc = tc.nc

    wpool = ctx.enter_context(tc.tile_pool(name="wpool", bufs=1))
    Wq_bf = wpool.tile([128, KC, DA], BF16, tag="wq")
    Wk_bf = wpool.tile([128, CC, DA], BF16, tag="wk")
    Wv_bf = wpool.tile([128, CC, DA], BF16, tag="wv")
    Wo_bf = wpool.tile([128, MC, FEAT], BF16, tag="wo")
    bo_bc = wpool.tile([128, FEAT], F32, tag="bo")
    ident = wpool.tile([128, 128], F32, tag="ident")
    # 0/1 picker: col 8 is ones; colpick[:, 8-h : 16-h] selects head h
    colpick = wpool.tile([128, 17], BF16, tag="colpick")
    # per head-pair hp: indb[2hp, hp, 0:64]=1, indb[2hp+1, hp, 64:128]=1
    indb = wpool.tile([8, HPAIRS, 128], BF16, tag="indb")

    for k in range(KC):
        nc.gpsimd.dma_start(out=Wq_bf[:, k, :], in_=Wq[128 * k : 128 * (k + 1), :])
    for c in range(CC):
        nc.gpsimd.dma_start(out=Wk_bf[:, c, :], in_=Wk[128 * c : 128 * (c + 1), :])
        nc.gpsimd.dma_start(out=Wv_bf[:, c, :], in_=Wv[128 * c : 128 * (c + 1), :])
    for m in range(MC):
        nc.gpsimd.dma_start(out=Wo_bf[:, m, :], in_=Wo[128 * m : 128 * (m + 1), :])
    bo_bcast_ap = bass.AP(tensor=bo.tensor, offset=bo.offset, ap=[[0, 128], *bo.ap])
    nc.gpsimd.dma_start(out=bo_bc[:, :], in_=bo_bcast_ap)
    nc.sync.dma_start(out=indb[:, :, :], in_=indb_d[:, :, :])
    make_identity(nc, ident)
    nc.gpsimd.memset(colpick[:, :], 0.0)
    nc.gpsimd.memset(colpick[:, 8:9], 1.0)

    # bf16 staging copy of x (transpose-loads below need a 2-byte dtype),
    # issued chunk-by-chunk in consumption order.
    for b in range(BP):
        for t in range(NT):
            nc.gpsimd.dma_start(
                out=x_bf[b, TC * t : TC * (t + 1), :],
                in_=x[b, TC * t : TC * (t + 1), :],
            )

    bpool = ctx.enter_context(tc.tile_pool(name="bpool", bufs=2))
    tpool = ctx.enter_context(tc.tile_pool(name="tpool", bufs=3))
    qpool = ctx.enter_context(tc.tile_pool(name="qpool", bufs=2))
    epool = ctx.enter_context(tc.tile_pool(name="epool", bufs=2))
    rpool = ctx.enter_context(tc.tile_pool(name="rpool", bufs=2))
    apool = ctx.enter_context(tc.tile_pool(name="apool", bufs=2))
    opool = ctx.enter_context(tc.tile_pool(name="opool", bufs=3))

    # PSUM: psQ 2 + psS 2 + psMB (sm+bcast) 2 + psOV (po+pu) 2 = 8 banks
    psQ = ctx.enter_context(tc.tile_pool(name="psQ", bufs=2, space="PSUM"))
    psS = ctx.enter_context(tc.tile_pool(name="psS", bufs=2, space="PSUM"))
    psMB = ctx.enter_context(tc.tile_pool(name="psMB", bufs=2, space="PSUM"))
    psOV = ctx.enter_context(tc.tile_pool(name="psOV", bufs=2, space="PSUM"))

    def prep_batch(b):
        """cond[b] -> cond.T -> kT / v projections."""
        cond_sb = bpool.tile([128, CD], F32, tag="cond", name=f"cond_{b}")
        nc.sync.dma_start(out=cond_sb[:M, :], in_=cond[b, :, :])
        condT = bpool.tile([128, CC, M], BF16, tag="condT", name=f"condT_{b}")
        for c in range(CC):
            ps = psQ.tile([128, TC], F32, tag="psQ", name=f"pst_{b}_{c}")
            nc.tensor.matmul(
                ps[:128, :M],
                cond_sb[:M, 128 * c : 128 * (c + 1)],
                ident[:M, :M],
                is_transpose=True,
            )
            nc.scalar.copy(condT[:, c, :], ps[:128, :M])
        kT = bpool.tile([128, MC, M], BF16, tag="kT", name=f"kT_{b}")
        for m in range(MC):
            pk = psQ.tile([128, TC], F32, tag="psQ", name=f"psk_{b}_{m}")
            for c in range(CC):
                nc.tensor.matmul(
                    pk[:, :M],
                    Wk_bf[:, c, 128 * m : 128 * (m + 1)],
                    condT[:, c, :],
                    start=(c == 0),
                    stop=(c == CC - 1),
                )
            nc.scalar.copy(kT[:, m, :], pk[:, :M])
        pv = psQ.tile([128, TC], F32, tag="psQ", name=f"psv_{b}")
        for c in range(CC):
            nc.tensor.matmul(
                pv[:M, :],
                condT[:, c, :],
                Wv_bf[:, c, :],
                start=(c == 0),
                stop=(c == CC - 1),
            )
        v_bf = bpool.tile([128, DA], BF16, tag="v", name=f"v_{b}")
        nc.scalar.copy(v_bf[:M, :], pv[:M, :])
        return kT, v_bf

    def emit_qT(g):
        """Transpose-load x chunk g and project: qT = Wq.T @ xT."""
        b, t = divmod(g, NT)
        tok0 = t * TC
        xT = tpool.tile([128, KC, TC], BF16, tag="xT", name=f"xT_{g}")
        for k in range(KC):
            nc.sync.dma_start(
                out=xT[:, k, :],
                in_=x_bf[b, tok0 : tok0 + TC, 128 * k : 128 * (k + 1)],
                transpose=True,
            )
        qT = qpool.tile([128, MC, TC], BF16, tag="qT", name=f"qT_{g}")
        for m in range(MC):
            pq = psQ.tile([128, TC], F32, tag="psQ", name=f"psq_{g}_{m}")
            for k in range(KC):
                nc.tensor.matmul(
                    pq,
                    Wq_bf[:, k, 128 * m : 128 * (m + 1)],
                    xT[:, k, :],
                    start=(k == 0),
                    stop=(k == KC - 1),
                )
            if m % 2 == 0:
                nc.vector.tensor_copy(qT[:, m, :], pq)
            else:
                nc.scalar.copy(qT[:, m, :], pq)
        return qT

    kT, v_bf = prep_batch(0)
    qT = emit_qT(0)

    for g in range(NG):
        b, t = divmod(g, NT)
        tok0 = t * TC

        # scores + exp (per head; even/odd heads run concurrently on
        # disjoint PE row groups into separate psum banks)
        E = epool.tile([128, H, TC], BF16, tag="E", name=f"E_{g}")
        for h in range(H):
            hp, r = divmod(h, 2)
            pse = psS.tile([128, TC], F32, tag="pse", name=f"pse_{g}_{h}")
            nc.tensor.matmul(
                pse[:M, :],
                kT[64 * r : 64 * r + 64, hp, :],
                qT[64 * r : 64 * r + 64, hp, :],
                start=True,
                stop=True,
            )
            nc.scalar.activation(E[:M, h, :], pse[:M, :], func=EXP, scale=DH**-0.5)

        # next chunk's projection: keeps the PE busy while exps run
        if g + 1 < NG:
            if (g + 1) % NT == 0:
                kT_next, v_next = prep_batch(b + 1)
            qT_next = emit_qT(g + 1)

        # per-head colsums accumulate into sm[8, tok] (emitted after qT(g+1);
        # by then the exps they wait on have drained)
        sm = psMB.tile([8, TC], F32, tag="mb", name=f"sm_{g}")
        for h in range(H):
            nc.tensor.matmul(
                sm,
                colpick[:M, 8 - h : 16 - h],
                E[:M, h, :],
                start=(h == 0),
                stop=(h == H - 1),
            )
        r8 = rpool.tile([8, TC], F32, tag="r8", name=f"r8_{g}")
        nc.vector.reciprocal_approx_fast(out=r8[:8, :], in_=sm[:8, :])
        r8b = rpool.tile([8, TC], BF16, tag="r8b", name=f"r8b_{g}")
        nc.vector.tensor_copy(r8b[:8, :], r8[:8, :])

        # attn @ v, normalized at copyback: aT[d_attn, tok]
        aT = apool.tile([128, MC, TC], BF16, tag="aT", name=f"aT_{g}")
        for hp in range(HPAIRS):
            psb = psMB.tile([128, TC], F32, tag="mb", name=f"psb_{g}_{hp}")
            nc.tensor.matmul(psb, indb[:8, hp, :], r8b[:8, :], start=True, stop=True)
            rs = rpool.tile([128, TC], F32, tag="rs", name=f"rs_{g}_{hp}")
            nc.scalar.copy(rs[:, :], psb[:, :])
            po = psOV.tile([128, TC], F32, tag="ov", name=f"po_{g}_{hp}")
            nc.tensor.matmul(
                po[0:64, :],
                v_bf[:M, 128 * hp : 128 * hp + 64],
                E[:M, 2 * hp, :],
                start=True,
                stop=True,
            )
            nc.tensor.matmul(
                po[64:128, :],
                v_bf[:M, 128 * hp + 64 : 128 * (hp + 1)],
                E[:M, 2 * hp + 1, :],
                start=True,
                stop=True,
            )
            nc.vector.tensor_mul(aT[:, hp, :], po[:, :], rs[:, :])

        # out = aT.T @ Wo + bo (bias added during psum -> sbuf copy)
        for s in range(SUB):
            pu = psOV.tile([128, FEAT], F32, tag="ov", name=f"pu_{g}_{s}")
            for m in range(MC):
                nc.tensor.matmul(
                    pu,
                    aT[:, m, 128 * s : 128 * (s + 1)],
                    Wo_bf[:, m, :],
                    start=(m == 0),
                    stop=(m == MC - 1),
                )
            osb = opool.tile([128, FEAT], F32, tag="osb", name=f"osb_{g}_{s}")
            nc.vector.tensor_add(osb, pu, bo_bc)
            nc.scalar.dma_start(
                out=out[b, tok0 + 128 * s : tok0 + 128 * (s + 1), :], in_=osb
            )

        if g + 1 < NG:
            if (g + 1) % NT == 0:
                kT, v_bf = kT_next, v_next
            qT = qT_next


def build():
    nc = bacc.Bacc(
        "TRN2", target_bir_lowering=False, debug=False, num_devices=N_CORES
    )
    x = nc.dram_tensor("x", [BP, N, FEAT], F32, kind="ExternalInput").ap()
    cond = nc.dram_tensor("cond", [BP, M, CD], F32, kind="ExternalInput").ap()
    Wq = nc.dram_tensor("Wq", [FEAT, DA], F32, kind="ExternalInput").ap()
    Wk = nc.dram_tensor("Wk", [CD, DA], F32, kind="ExternalInput").ap()
    Wv = nc.dram_tensor("Wv", [CD, DA], F32, kind="ExternalInput").ap()
    Wo = nc.dram_tensor("Wo", [DA, FEAT], F32, kind="ExternalInput").ap()
    bo = nc.dram_tensor("bo", [FEAT], F32, kind="ExternalInput").ap()
    indb_d = nc.dram_tensor("indb", [8, HPAIRS, 128], BF16, kind="ExternalInput").ap()
    out = nc.dram_tensor("out", [BP, N, FEAT], F32, kind="ExternalOutput").ap()
    x_bf = nc.dram_tensor("x_bf16_stage", [BP, N, FEAT], BF16).ap()
    with tile.TileContext(nc) as tc:
        _body(tc, x, x_bf, cond, Wq, Wk, Wv, Wo, bo, indb_d, out)
    nc.compile()
    return nc


_NC = None


def kernel(x, cond, Wq, Wk, Wv, Wo, bo, _trace=False):
    global _NC
    if _NC is None:
        _NC = build()
    import ml_dtypes

    indb_np = np.zeros((8, HPAIRS, 128), dtype=ml_dtypes.bfloat16)
    for hp in range(HPAIRS):
        indb_np[2 * hp, hp, 0:64] = 1
        indb_np[2 * hp + 1, hp, 64:128] = 1
    shared = {
        "Wq": np.asarray(Wq, np.float32),
        "Wk": np.asarray(Wk, np.float32),
        "Wv": np.asarray(Wv, np.float32),
        "Wo": np.asarray(Wo, np.float32),
        "bo": np.asarray(bo, np.float32),
        "indb": indb_np,
    }
    in_maps = [
        {
            "x": np.ascontiguousarray(x[BP * i : BP * (i + 1)], dtype=np.float32),
            "cond": np.ascontiguousarray(cond[BP * i : BP * (i + 1)], dtype=np.float32),
            **shared,
        }
        for i in range(N_CORES)
    ]
    res = run_bass_kernel_spmd(_NC, in_maps, list(range(N_CORES)), trace=_trace)
    out = np.concatenate([r["out"] for r in res.results], axis=0)
    if _trace:
        kernel.last_exec_time_ns = res.exec_time_ns
        kernel.last_results = res
    return out


# revision 13
# speedup vs baseline: 1.2623x; 1.0358x over previous
"""Cross-attention block (B=16, N=4096 queries, M=77 keys, 8 heads x 64) on 8 trn2 cores.

Sharding: data-parallel over batch; each core gets 2 batches, full weights.

Per-core dataflow (matmuls bf16 in / fp32 psum), software-pipelined so the PE
always has independent work (HAM stays warm):
  x -> bf16 staging copy in DRAM (gpsimd cast DMA), xbar-transposed per chunk
  into xT [feat, tok];  qT = Wq.T @ xT.
  Per chunk g the emission order is: scores(g) -> qT(g+1) -> colsums(g) ->
  recip -> bcast/attnv/normalize(g) -> out(g), so qT(g+1) matmuls fill the
  PE while the scalar engine runs the exps of chunk g.
  Softmax denominators via indicator-matmul colsums; reciprocals broadcast
  across partitions with a K=8 indicator matmul (all on-chip, no DRAM bounce);
  aT = (v_h.T @ E_h) * recip with a both-PSUM DVE multiply.
  out = aT.T @ Wo + bo (bias added during the PSUM->SBUF copy on DVE).
"""

import numpy as np

import concourse.bass as bass
import concourse.mybir as mybir
import concourse.tile as tile
from concourse import bacc
from concourse._compat import with_exitstack
from concourse.bass_utils import run_bass_kernel_spmd
from concourse.masks import make_identity
from contextlib import ExitStack

N_CORES = 8
B, N, FEAT, CD = 16, 4096, 512, 768
M = 77          # cond tokens
H, DH = 8, 64
DA = H * DH     # 512
BP = B // N_CORES   # batches per core
TC = 512            # token chunk
NT = N // TC        # chunks per batch
NG = BP * NT        # chunks per core
SUB = TC // 128     # 128-token subtiles per chunk
KC = FEAT // 128    # x feature chunks
CC = CD // 128      # cond feature chunks
MC = DA // 128      # d_attn chunks
HPAIRS = H // 2

F32 = mybir.dt.float32
BF16 = mybir.dt.bfloat16
EXP = mybir.ActivationFunctionType.Exp


@with_exitstack
def _body(
    ctx: ExitStack, tc: tile.TileContext, x, x_bf, cond, Wq, Wk, Wv, Wo, bo, indb_d, out
):
    nc = tc.nc

    wpool = ctx.enter_context(tc.tile_pool(name="wpool", bufs=1))
    Wq_bf = wpool.tile([128, KC, DA], BF16, tag="wq")
    Wk_bf = wpool.tile([128, CC, DA], BF16, tag="wk")
    Wv_bf = wpool.tile([128, CC, DA], BF16, tag="wv")
    Wo_bf = wpool.tile([128, MC, FEAT], BF16, tag="wo")
    bo_bc = wpool.tile([128, FEAT], F32, tag="bo")
    ident = wpool.tile([128, 128], F32, tag="ident")
    # 0/1 picker: col 8 is ones; colpick[:, 8-h : 16-h] selects head h
    colpick = wpool.tile([128, 17], BF16, tag="colpick")
    # per head-pair hp: indb[2hp, hp, 0:64]=1, indb[2hp+1, hp, 64:128]=1
    indb = wpool.tile([8, HPAIRS, 128], BF16, tag="indb")

    for k in range(KC):
        nc.gpsimd.dma_start(out=Wq_bf[:, k, :], in_=Wq[128 * k : 128 * (k + 1), :])
    for c in range(CC):
        nc.gpsimd.dma_start(out=Wk_bf[:, c, :], in_=Wk[128 * c : 128 * (c + 1), :])
        nc.gpsimd.dma_start(out=Wv_bf[:, c, :], in_=Wv[128 * c : 128 * (c + 1), :])
    for m in range(MC):
        nc.gpsimd.dma_start(out=Wo_bf[:, m, :], in_=Wo[128 * m : 128 * (m + 1), :])
    bo_bcast_ap = bass.AP(tensor=bo.tensor, offset=bo.offset, ap=[[0, 128], *bo.ap])
    nc.gpsimd.dma_start(out=bo_bc[:, :], in_=bo_bcast_ap)
    nc.sync.dma_start(out=indb[:, :, :], in_=indb_d[:, :, :])
    make_identity(nc, ident)
    nc.gpsimd.memset(colpick[:, :], 0.0)
    nc.gpsimd.memset(colpick[:, 8:9], 1.0)

    # bf16 staging copy of x (transpose-loads below need a 2-byte dtype),
    # issued chunk-by-chunk in consumption order.
    for b in range(BP):
        for t in range(NT):
            nc.gpsimd.dma_start(
                out=x_bf[b, TC * t : TC * (t + 1), :],
                in_=x[b, TC * t : TC * (t + 1), :],
            )

    bpool = ctx.enter_context(tc.tile_pool(name="bpool", bufs=2))
    tpool = ctx.enter_context(tc.tile_pool(name="tpool", bufs=3))
    qpool = ctx.enter_context(tc.tile_pool(name="qpool", bufs=2))
    epool = ctx.enter_context(tc.tile_pool(name="epool", bufs=2))
    rpool = ctx.enter_context(tc.tile_pool(name="rpool", bufs=2))
    apool = ctx.enter_context(tc.tile_pool(name="apool", bufs=2))
    opool = ctx.enter_context(tc.tile_pool(name="opool", bufs=3))

    # PSUM: psQ 2 + psS 2 + psMB (sm+bcast) 2 + psOV (po+pu) 2 = 8 banks
    psQ = ctx.enter_context(tc.tile_pool(name="psQ", bufs=2, space="PSUM"))
    psS = ctx.enter_context(tc.tile_pool(name="psS", bufs=2, space="PSUM"))
    psMB = ctx.enter_context(tc.tile_pool(name="psMB", bufs=2, space="PSUM"))
    psOV = ctx.enter_context(tc.tile_pool(name="psOV", bufs=2, space="PSUM"))

    def prep_batch(b):
        """cond[b] -> cond.T -> kT / v projections."""
        cond_sb = bpool.tile([128, CD], F32, tag="cond", name=f"cond_{b}")
        nc.sync.dma_start(out=cond_sb[:M, :], in_=cond[b, :, :])
        condT = bpool.tile([128, CC, M], BF16, tag="condT", name=f"condT_{b}")
        for c in range(CC):
            ps = psQ.tile([128, TC], F32, tag="psQ", name=f"pst_{b}_{c}")
            nc.tensor.matmul(
                ps[:128, :M],
                cond_sb[:M, 128 * c : 128 * (c + 1)],
                ident[:M, :M],
                is_transpose=True,
            )
            nc.scalar.copy(condT[:, c, :], ps[:128, :M])
        kT = bpool.tile([128, MC, M], BF16, tag="kT", name=f"kT_{b}")
        for m in range(MC):
            pk = psQ.tile([128, TC], F32, tag="psQ", name=f"psk_{b}_{m}")
            for c in range(CC):
                nc.tensor.matmul(
                    pk[:, :M],
                    Wk_bf[:, c, 128 * m : 128 * (m + 1)],
                    condT[:, c, :],
                    start=(c == 0),
                    stop=(c == CC - 1),
                )
            nc.scalar.copy(kT[:, m, :], pk[:, :M])
        pv = psQ.tile([128, TC], F32, tag="psQ", name=f"psv_{b}")
        for c in range(CC):
            nc.tensor.matmul(
                pv[:M, :],
                condT[:, c, :],
                Wv_bf[:, c, :],
                start=(c == 0),
                stop=(c == CC - 1),
            )
        v_bf = bpool.tile([128, DA], BF16, tag="v", name=f"v_{b}")
        nc.scalar.copy(v_bf[:M, :], pv[:M, :])
        return kT, v_bf

    def emit_qT(g):
        """Transpose-load x chunk g and project: qT = Wq.T @ xT."""
        b, t = divmod(g, NT)
        tok0 = t * TC
        xT = tpool.tile([128, KC, TC], BF16, tag="xT", name=f"xT_{g}")
        for k in range(KC):
            nc.sync.dma_start(
                out=xT[:, k, :],
                in_=x_bf[b, tok0 : tok0 + TC, 128 * k : 128 * (k + 1)],
                transpose=True,
            )
        qT = qpool.tile([128, MC, TC], BF16, tag="qT", name=f"qT_{g}")
        for m in range(MC):
            pq = psQ.tile([128, TC], F32, tag="psQ", name=f"psq_{g}_{m}")
            for k in range(KC):
                nc.tensor.matmul(
                    pq,
                    Wq_bf[:, k, 128 * m : 128 * (m + 1)],
                    xT[:, k, :],
                    start=(k == 0),
                    stop=(k == KC - 1),
                )
            if m % 2 == 0:
                nc.vector.tensor_copy(qT[:, m, :], pq)
            else:
                nc.scalar.copy(qT[:, m, :], pq)
        return qT

    def emit_xT(g):
        b, t = divmod(g, NT)
        tok0 = t * TC
        xT = tpool.tile([128, KC, TC], BF16, tag="xT", name=f"xTt_{g}")
        for k in range(KC):
            nc.sync.dma_start(
                out=xT[:, k, :],
                in_=x_bf[b, tok0 : tok0 + TC, 128 * k : 128 * (k + 1)],
                transpose=True,
            )
        return xT

    def emit_proj(g, xT):
        """qT = Wq.T @ xT for chunk g (xT transpose-loaded earlier)."""
        qT = qpool.tile([128, MC, TC], BF16, tag="qT", name=f"qT_{g}")
        for m in range(MC):
            pq = psQ.tile([128, TC], F32, tag="psQ", name=f"psq_{g}_{m}")
            for k in range(KC):
                nc.tensor.matmul(
                    pq,
                    Wq_bf[:, k, 128 * m : 128 * (m + 1)],
                    xT[:, k, :],
                    start=(k == 0),
                    stop=(k == KC - 1),
                )
            if m % 2 == 0:
                nc.vector.tensor_copy(qT[:, m, :], pq)
            else:
                nc.scalar.copy(qT[:, m, :], pq)
        return qT

    def emit_score_pair(g, hp, kT_s, qT_s, E):
        """Two row-tiled score matmuls (concurrent on PE) + their exps."""
        for r in range(2):
            h = 2 * hp + r
            pse = psS.tile([128, TC], F32, tag="pse", name=f"pse_{g}_{h}")
            nc.tensor.matmul(
                pse[:M, :],
                kT_s[64 * r : 64 * r + 64, hp, :],
                qT_s[64 * r : 64 * r + 64, hp, :],
                start=True,
                stop=True,
            )
            nc.scalar.activation(E[:M, h, :], pse[:M, :], func=EXP, scale=DH**-0.5)

    # software-pipeline prologue
    kT, v_bf = prep_batch(0)
    kT_next, v_next = kT, v_bf
    xT0 = emit_xT(0)
    xT1 = emit_xT(1)
    xTs = {0: xT0, 1: xT1}
    qTs = {0: emit_proj(0, xT0), 1: emit_proj(1, xT1)}
    Es = {0: epool.tile([128, H, TC], BF16, tag="E", name="E_0")}
    for hp in range(HPAIRS):
        emit_score_pair(0, hp, kT, qTs[0], Es[0])

    for g in range(NG):
        b, t = divmod(g, NT)
        tok0 = t * TC
        E = Es.pop(g)

        # chunk g+2's transpose loads (prefetch)
        if g + 2 < NG:
            xTs[g + 2] = emit_xT(g + 2)

        # per-head colsums into sm[8, tok] (exps of chunk g drained last iter)
        sm = psMB.tile([8, TC], F32, tag="mb", name=f"sm_{g}")
        for h in range(H):
            nc.tensor.matmul(
                sm,
                colpick[:M, 8 - h : 16 - h],
                E[:M, h, :],
                start=(h == 0),
                stop=(h == H - 1),
            )
        r8 = rpool.tile([8, TC], F32, tag="r8", name=f"r8_{g}")
        nc.vector.reciprocal_approx_fast(out=r8[:8, :], in_=sm[:8, :])
        r8b = rpool.tile([8, TC], BF16, tag="r8b", name=f"r8b_{g}")
        nc.vector.tensor_copy(r8b[:8, :], r8[:8, :])

        if g + 1 < NG and (g + 1) % NT == 0:
            kT_next, v_next = prep_batch(b + 1)

        # interleave: scores(g+1) pairs woven between bcast/attnv(g) pairs so
        # the PE never idles on the recip chain or the exp pacing
        aT = apool.tile([128, MC, TC], BF16, tag="aT", name=f"aT_{g}")
        if g + 1 < NG:
            Es[g + 1] = epool.tile([128, H, TC], BF16, tag="E", name=f"E_{g+1}")
        for hp in range(HPAIRS):
            if g + 1 < NG:
                emit_score_pair(g + 1, hp, kT_next, qTs[g + 1], Es[g + 1])
            psb = psMB.tile([128, TC], F32, tag="mb", name=f"psb_{g}_{hp}")
            nc.tensor.matmul(psb, indb[:8, hp, :], r8b[:8, :], start=True, stop=True)
            rs = rpool.tile([128, TC], F32, tag="rs", name=f"rs_{g}_{hp}")
            nc.vector.tensor_copy(rs[:, :], psb[:, :])
            po = psOV.tile([128, TC], F32, tag="ov", name=f"po_{g}_{hp}")
            nc.tensor.matmul(
                po[0:64, :],
                v_bf[:M, 128 * hp : 128 * hp + 64],
                E[:M, 2 * hp, :],
                start=True,
                stop=True,
            )
            nc.tensor.matmul(
                po[64:128, :],
                v_bf[:M, 128 * hp + 64 : 128 * (hp + 1)],
                E[:M, 2 * hp + 1, :],
                start=True,
                stop=True,
            )
            nc.vector.tensor_mul(aT[:, hp, :], po[:, :], rs[:, :])

        # qT(g+2): fills the PE while the aT multiplies drain on DVE
        if g + 2 < NG:
            qTs[g + 2] = emit_proj(g + 2, xTs.pop(g + 2))

        # out = aT.T @ Wo + bo (bias added during psum -> sbuf copy)
        for s in range(SUB):
            pu = psOV.tile([128, FEAT], F32, tag="ov", name=f"pu_{g}_{s}")
            for m in range(MC):
                nc.tensor.matmul(
                    pu,
                    aT[:, m, 128 * s : 128 * (s + 1)],
                    Wo_bf[:, m, :],
                    start=(m == 0),
                    stop=(m == MC - 1),
                )
            osb = opool.tile([128, FEAT], F32, tag="osb", name=f"osb_{g}_{s}")
            nc.vector.tensor_add(osb, pu, bo_bc)
            nc.sync.dma_start(
                out=out[b, tok0 + 128 * s : tok0 + 128 * (s + 1), :], in_=osb
            )

        qTs.pop(g, None)
        xTs.pop(g, None)
        if g + 1 < NG and (g + 1) % NT == 0:
            kT, v_bf = kT_next, v_next


def build():
    nc = bacc.Bacc(
        "TRN2", target_bir_lowering=False, debug=False, num_devices=N_CORES
    )
    x = nc.dram_tensor("x", [BP, N, FEAT], F32, kind="ExternalInput").ap()
    cond = nc.dram_tensor("cond", [BP, M, CD], F32, kind="ExternalInput").ap()
    Wq = nc.dram_tensor("Wq", [FEAT, DA], F32, kind="ExternalInput").ap()
    Wk = nc.dram_tensor("Wk", [CD, DA], F32, kind="ExternalInput").ap()
    Wv = nc.dram_tensor("Wv", [CD, DA], F32, kind="ExternalInput").ap()
    Wo = nc.dram_tensor("Wo", [DA, FEAT], F32, kind="ExternalInput").ap()
    bo = nc.dram_tensor("bo", [FEAT], F32, kind="ExternalInput").ap()
    indb_d = nc.dram_tensor("indb", [8, HPAIRS, 128], BF16, kind="ExternalInput").ap()
    out = nc.dram_tensor("out", [BP, N, FEAT], F32, kind="ExternalOutput").ap()
    x_bf = nc.dram_tensor("x_bf16_stage", [BP, N, FEAT], BF16).ap()
    with tile.TileContext(nc) as tc:
        _body(tc, x, x_bf, cond, Wq, Wk, Wv, Wo, bo, indb_d, out)
    nc.compile()
    return nc


_NC = None


def kernel(x, cond, Wq, Wk, Wv, Wo, bo, _trace=False):
    global _NC
    if _NC is None:
        _NC = build()
    import ml_dtypes

    indb_np = np.zeros((8, HPAIRS, 128), dtype=ml_dtypes.bfloat16)
    for hp in range(HPAIRS):
        indb_np[2 * hp, hp, 0:64] = 1
        indb_np[2 * hp + 1, hp, 64:128] = 1
    shared = {
        "Wq": np.asarray(Wq, np.float32),
        "Wk": np.asarray(Wk, np.float32),
        "Wv": np.asarray(Wv, np.float32),
        "Wo": np.asarray(Wo, np.float32),
        "bo": np.asarray(bo, np.float32),
        "indb": indb_np,
    }
    in_maps = [
        {
            "x": np.ascontiguousarray(x[BP * i : BP * (i + 1)], dtype=np.float32),
            "cond": np.ascontiguousarray(cond[BP * i : BP * (i + 1)], dtype=np.float32),
            **shared,
        }
        for i in range(N_CORES)
    ]
    res = run_bass_kernel_spmd(_NC, in_maps, list(range(N_CORES)), trace=_trace)
    out = np.concatenate([r["out"] for r in res.results], axis=0)
    if _trace:
        kernel.last_exec_time_ns = res.exec_time_ns
        kernel.last_results = res
    return out
